# revision 7
# baseline (speedup 1.0000x reference)
"""Trainium2 Bass kernel for nn_Coo2Cel (periodic pairwise displacement grid),
compact min-image formulation.

With box = 30 > 2*rc = 12, at most ONE of the 27 lattice shifts (the minimum
image, sigma_c = round(d_c/box)) can fall inside the cutoff for any pair, so
the dense [N,N,27,4] output is >96% structural zeros. The device computes,
for each pair (i,j):
    w_c  = d_c - box*round(d_c/box)   (bit-identical to the reference's
                                       d_c - sigma_c*box at the passing shift)
    sod  = wx^2 + wy^2 + wz^2
    mask = sod < rc^2
    out planes: (mask*wx, mask*wy, mask*wz, mask*sod, sidx)
with sidx = 9*sx + 3*sy + sz, s_c = sigma_c + 1 — the flat shift index of the
minimum image. The host assembles the full [1,N,N,27,4] by placing each
pair's 4-vector at its sidx slot in a zeros array (pairs that fail the cutoff
scatter zeros onto a slot the reference also leaves zero, so no masking of
sidx is needed; the self pair has d = 0 exactly and scatters zeros at the
zero shift, which the reference zeroes too).

round() is computed exactly on the scalar engine with the f32 magic-constant
trick: Identity(d*(1/box) + 1.5*2^23) - 1.5*2^23 (ties-to-even; ties occur
only at |d_c| = box/2 where both images fail the cutoff, so the choice is
immaterial).

Sharding: query rows i split row-wise across 8 NeuronCores (128 rows per
core = the 128 SBUF partitions); every core holds all N candidates.

Engine split (per 1024-candidate row, f32):
  DVE   : d (tensor_scalar 2x), sigma (tensor_scalar 2x), w (STT),
          mask planes 0:2 (STT)
  ACT   : t = magic-round, squares, 9*sx+13 (Identity/Square, chunk-sized)
  Pool  : sod adds, mask planes 2:4 (STT), sidx assembly
  SP    : output DMAs; SWDGE (gpsimd queue) input DMA
"""
import sys

if "/opt/trn_rl_repo" not in sys.path:
    sys.path.insert(0, "/opt/trn_rl_repo")

import numpy as np

N = 1024          # atoms
S = 27            # lattice shifts
P = 128           # partitions / query rows per core
NCORES = 8
RC2 = 36.0        # rc^2, rc = 6.0
MAGIC = 12582912.0  # 1.5 * 2^23: float32 round-to-nearest-int bias

TRACE = False
LAST_RESULT = None

_CACHE = {}

# engine assignment / tiling knobs (module-level so experiments can tweak)
# NOTE: the Pool engine only accepts tensor_tensor/tensor_copy (TensorScalarPtr
# fails the V3 ISA check), so everything Pool-side is expressed as TT.
CFG = dict(
    nch=4,              # candidate chunks per rep (C = N/nch)
    work_bufs=3,        # work-pool depth (pipeline across chunks/reps)
    out_bufs=4,         # out-tile pool depth
    d_dve=False,        # d on ACT Identity (small values, table-safe)
    use_pool=False,     # Pool TT chain is slow on HW (Q7 software impl)
    pool_sod=True,      # ...but the two sod adds fit under its throughput
    out_f32=False,      # f32 output planes (False: bf16)
    hoist_in=True,      # load the candidate table once (baseline convention)
)
import os as _os, json as _json
CFG.update(_json.loads(_os.environ.get("K2_CFG", "{}")))


def _build(box, pbc_tuple, repeat=1, cfg=None):
    cfg = {**CFG, **(cfg or {})}
    nch = cfg["nch"]
    C = N // nch
    import concourse.bacc as bacc
    import concourse.mybir as mybir
    from concourse.tile import TileContext

    F32 = mybir.dt.float32
    OUTDT = F32 if cfg["out_f32"] else mybir.dt.bfloat16
    ADD = mybir.AluOpType.add
    MULT = mybir.AluOpType.mult
    SUB = mybir.AluOpType.subtract
    ISLT = mybir.AluOpType.is_lt
    IDENT = mybir.ActivationFunctionType.Identity
    SQUARE = mybir.ActivationFunctionType.Square

    assert box[0] == box[1] == box[2], "kernel assumes a cubic box"
    bx = float(box[0])

    nc = bacc.Bacc()
    pin_d = nc.declare_dram_parameter("pin", [P, 3, N + 1], F32, isOutput=False)
    out_d = nc.declare_dram_parameter("out", [P, 5, N], OUTDT, isOutput=True)

    pool_eng = nc.gpsimd if cfg["use_pool"] else nc.vector

    with TileContext(nc) as tc:
        with (
            tc.tile_pool(name="constp", bufs=1) as cpool,
            tc.tile_pool(name="pinp", bufs=2) as ppool,
            tc.tile_pool(name="work", bufs=cfg["work_bufs"]) as wpool,
            tc.tile_pool(name="outp", bufs=cfg["out_bufs"]) as opool,
        ):
            if cfg["use_pool"]:
                C3 = cpool.tile([P, 1], F32)
                nc.vector.memset(C3[:], 3.0)
            if cfg["hoist_in"]:
                pin = ppool.tile([P, 3, N + 1], F32, tag="pin")
                nc.gpsimd.dma_start(out=pin[:], in_=pin_d[:])
            for rep in range(repeat):
                if not cfg["hoist_in"]:
                    pin = ppool.tile([P, 3, N + 1], F32, tag="pin")
                    # input on the SWDGE (gpsimd) queue: keeps the HWDGE
                    # lanes free for output DMAs, overlapping previous rep
                    nc.gpsimd.dma_start(out=pin[:], in_=pin_d[:])
                q = [pin[:, c, N:N + 1] for c in range(3)]

                for ch in range(nch):
                    js = slice(ch * C, (ch + 1) * C)
                    D = wpool.tile([P, 3, C], F32, tag="D")
                    T = wpool.tile([P, 3, C], F32, tag="T")
                    SG = wpool.tile([P, 3, C], F32, tag="SG")
                    SQ = wpool.tile([P, 3, C], F32, tag="SQ")
                    W = wpool.tile([P, 4, C], F32, tag="W")
                    S0 = wpool.tile([P, C], F32, tag="S0")
                    U = wpool.tile([P, C], F32, tag="U")
                    V = wpool.tile([P, C], F32, tag="V")
                    if cfg["use_pool"]:
                        M = wpool.tile([P, C], F32, tag="M")
                        V2 = wpool.tile([P, C], F32, tag="V2")
                    outt = opool.tile([P, 5, C], OUTDT, tag="outt")

                    # d_c = q_c - p_c  (ACT Identity: affine, small values)
                    for c in range(3):
                        if cfg["d_dve"]:
                            nc.vector.tensor_scalar(
                                out=D[:, c, :], in0=pin[:, c, js],
                                scalar1=q[c], scalar2=-1.0,
                                op0=SUB, op1=MULT)
                        else:
                            nc.scalar.activation(
                                out=D[:, c, :], in_=pin[:, c, js],
                                func=IDENT, bias=q[c], scale=-1.0)
                    # t = d/box + MAGIC: f32 addition rounds to
                    # round(d/box) + MAGIC exactly (DVE ALU, bit-exact)
                    nc.vector.tensor_scalar(
                        out=T[:], in0=D[:], scalar1=1.0 / bx, scalar2=MAGIC,
                        op0=MULT, op1=ADD)
                    # sigma = t - MAGIC in {-1, 0, +1}
                    nc.vector.tensor_scalar(
                        out=SG[:], in0=T[:], scalar1=MAGIC, scalar2=None,
                        op0=SUB)
                    # non-periodic axes: only the zero shift is allowed
                    for c in range(3):
                        if not pbc_tuple[c]:
                            nc.vector.memset(SG[:, c, :], 0.0)
                    # w = d - box*sigma  (min-image displacement)
                    nc.vector.scalar_tensor_tensor(
                        out=W[:, 0:3, :], in0=SG[:], scalar=-bx, in1=D[:],
                        op0=MULT, op1=ADD)
                    # squares on ACT
                    nc.scalar.activation(out=SQ[:], in_=W[:, 0:3, :],
                                         func=SQUARE)
                    # sod = (sqx + sqy) + sqz — the one Pool-sized job
                    sod_eng = nc.gpsimd if cfg["pool_sod"] else pool_eng
                    sod_eng.tensor_tensor(
                        out=S0[:], in0=SQ[:, 0, :], in1=SQ[:, 1, :], op=ADD)
                    sod_eng.tensor_tensor(
                        out=W[:, 3, :], in0=S0[:], in1=SQ[:, 2, :], op=ADD)

                    # first-writer memset absorbs the DMA slot-recycling wait
                    nc.vector.memset(outt[:, 0, 0:1], 0.0)
                    if cfg["use_pool"]:
                        # masked planes = m * [wx, wy, wz, sod] (one Pool TT)
                        nc.vector.tensor_scalar(
                            out=M[:], in0=W[:, 3, :], scalar1=RC2,
                            scalar2=None, op0=ISLT)
                        pool_eng.tensor_tensor(
                            out=outt[:, 0:4, :], in0=W[:, 0:4, :],
                            in1=M[:].unsqueeze(1).broadcast_to([P, 4, C]),
                            op=MULT)
                        nc.vector.tensor_scalar(
                            out=U[:], in0=SG[:, 0, :], scalar1=9.0,
                            scalar2=13.0, op0=MULT, op1=ADD)
                        pool_eng.tensor_tensor(
                            out=V[:], in0=SG[:, 1, :],
                            in1=C3[:].broadcast_to([P, C]), op=MULT)
                        pool_eng.tensor_tensor(
                            out=V2[:], in0=V[:], in1=SG[:, 2, :], op=ADD)
                        pool_eng.tensor_tensor(
                            out=outt[:, 4, :], in0=U[:], in1=V2[:], op=ADD)
                    else:
                        # fused masked planes: (sod < rc^2) * plane, one STT
                        nc.vector.scalar_tensor_tensor(
                            out=outt[:, 0:4, :],
                            in0=W[:, 3:4, :].broadcast_to([P, 4, C]),
                            scalar=RC2, in1=W[:, 0:4, :],
                            op0=ISLT, op1=MULT)
                        # sidx = (9*sx + 13) + (3*sy + sz)
                        nc.vector.tensor_scalar(
                            out=U[:], in0=SG[:, 0, :], scalar1=9.0,
                            scalar2=13.0, op0=MULT, op1=ADD)
                        nc.vector.scalar_tensor_tensor(
                            out=V[:], in0=SG[:, 1, :], scalar=3.0,
                            in1=SG[:, 2, :], op0=MULT, op1=ADD)
                        nc.vector.tensor_tensor(
                            out=outt[:, 4, :], in0=U[:], in1=V[:], op=ADD)

                    nc.sync.dma_start(out=out_d[:, :, js], in_=outt[:])
    nc.finalize()
    return nc


def _prep(pos_cel, cel_mat, pbc):
    """Host-side prep shared by kernel() and bench.py."""
    pos_cel = np.asarray(pos_cel)
    cel_mat = np.asarray(cel_mat, dtype=np.float32)
    pbc = np.asarray(pbc)
    B = pos_cel.shape[0]
    assert pos_cel.shape == (B, N, 3), pos_cel.shape
    assert B == 1

    pos = pos_cel[0].astype(np.float32) @ cel_mat[0]
    pos = pos.astype(np.float32)
    off = cel_mat[0] - np.diag(np.diag(cel_mat[0]))
    assert np.all(off == 0), "kernel assumes a diagonal cell matrix"
    box = tuple(float(cel_mat[0][c, c]) for c in range(3))
    pbc_tuple = tuple(bool(x) for x in pbc[0])

    in_maps = []
    for k in range(NCORES):
        pin = np.empty((P, 3, N + 1), dtype=np.float32)
        pin[:, :, :N] = pos.T[None]
        pin[:, :, N] = pos[k * P:(k + 1) * P]
        in_maps.append({"pin": pin})
    return box, pbc_tuple, in_maps


def _scatter(planes):
    """planes: [N, 5, N] device output -> full [1, N, N, S, 4] f32."""
    vals = np.ascontiguousarray(
        planes[:, 0:4, :].transpose(0, 2, 1), dtype=np.float32)  # [N, N, 4]
    sidx = np.clip(planes[:, 4, :].astype(np.int32), 0, S - 1)   # [N, N]
    out = np.zeros((N * N * S, 4), np.float32)
    ii = np.arange(N, dtype=np.int64)
    lin = (ii[:, None] * N + ii[None, :]) * S + sidx
    out[lin.reshape(-1)] = vals.reshape(N * N, 4)
    return out.reshape(1, N, N, S, 4)


def kernel(pos_cel, cel_mat, pbc):
    global LAST_RESULT
    from concourse.bass_utils import run_bass_kernel_spmd

    box, pbc_tuple, in_maps = _prep(pos_cel, cel_mat, pbc)

    key = (box, pbc_tuple)
    if key not in _CACHE:
        _CACHE[key] = _build(box, pbc_tuple)
    nc = _CACHE[key]

    res = run_bass_kernel_spmd(nc, in_maps, list(range(NCORES)), trace=TRACE)
    LAST_RESULT = res

    planes = np.concatenate(
        [np.asarray(res.results[k]["out"], dtype=np.float32)
         for k in range(NCORES)], axis=0)              # [N, 5, N]
    return _scatter(planes)


# revision 8
# speedup vs baseline: 1.0627x; 1.0627x over previous
"""Trainium2 Bass kernel for nn_Coo2Cel (periodic pairwise displacement grid),
compact min-image formulation.

With box = 30 > 2*rc = 12, at most ONE of the 27 lattice shifts (the minimum
image, sigma_c = round(d_c/box)) can fall inside the cutoff for any pair, so
the dense [N,N,27,4] output is >96% structural zeros. The device computes,
for each pair (i,j):
    w_c  = d_c - box*round(d_c/box)   (bit-identical to the reference's
                                       d_c - sigma_c*box at the passing shift)
    sod  = wx^2 + wy^2 + wz^2
    mask = sod < rc^2
    out planes: (mask*wx, mask*wy, mask*wz, mask*sod, sidx)
with sidx = 9*sx + 3*sy + sz, s_c = sigma_c + 1 — the flat shift index of the
minimum image. The host assembles the full [1,N,N,27,4] by placing each
pair's 4-vector at its sidx slot in a zeros array (pairs that fail the cutoff
scatter zeros onto a slot the reference also leaves zero, so no masking of
sidx is needed; the self pair has d = 0 exactly and scatters zeros at the
zero shift, which the reference zeroes too).

round() is computed exactly on the scalar engine with the f32 magic-constant
trick: Identity(d*(1/box) + 1.5*2^23) - 1.5*2^23 (ties-to-even; ties occur
only at |d_c| = box/2 where both images fail the cutoff, so the choice is
immaterial).

Sharding: query rows i split row-wise across 8 NeuronCores (128 rows per
core = the 128 SBUF partitions); every core holds all N candidates.

Engine split (per 1024-candidate row, f32):
  DVE   : d (tensor_scalar 2x), sigma (tensor_scalar 2x), w (STT),
          mask planes 0:2 (STT)
  ACT   : t = magic-round, squares, 9*sx+13 (Identity/Square, chunk-sized)
  Pool  : sod adds, mask planes 2:4 (STT), sidx assembly
  SP    : output DMAs; SWDGE (gpsimd queue) input DMA
"""
import sys

if "/opt/trn_rl_repo" not in sys.path:
    sys.path.insert(0, "/opt/trn_rl_repo")

import numpy as np

N = 1024          # atoms
S = 27            # lattice shifts
P = 128           # partitions / query rows per core
NCORES = 8
RC2 = 36.0        # rc^2, rc = 6.0
MAGIC = 12582912.0  # 1.5 * 2^23: float32 round-to-nearest-int bias

TRACE = False
LAST_RESULT = None

_CACHE = {}

# engine assignment / tiling knobs (module-level so experiments can tweak)
# NOTE: the Pool engine only accepts tensor_tensor/tensor_copy (TensorScalarPtr
# fails the V3 ISA check), so everything Pool-side is expressed as TT.
CFG = dict(
    nch=2,              # candidate chunks per rep (C = N/nch)
    work_bufs=2,        # work-pool depth (pipeline across chunks/reps)
    out_bufs=3,         # out-tile pool depth
    d_dve=False,        # d on ACT Identity (small values, table-safe)
    use_pool=False,     # Pool TT chain is slow on HW (Q7 software impl)
    pool_sod=True,      # ...but the two sod adds fit under its throughput
    out_f32=False,      # f32 output planes (False: bf16)
    hoist_in=True,      # load the candidate table once (baseline convention)
)
import os as _os, json as _json
CFG.update(_json.loads(_os.environ.get("K2_CFG", "{}")))


def _build(box, pbc_tuple, repeat=1, cfg=None):
    cfg = {**CFG, **(cfg or {})}
    nch = cfg["nch"]
    C = N // nch
    import concourse.bacc as bacc
    import concourse.mybir as mybir
    from concourse.tile import TileContext

    F32 = mybir.dt.float32
    OUTDT = F32 if cfg["out_f32"] else mybir.dt.bfloat16
    ADD = mybir.AluOpType.add
    MULT = mybir.AluOpType.mult
    SUB = mybir.AluOpType.subtract
    ISLT = mybir.AluOpType.is_lt
    IDENT = mybir.ActivationFunctionType.Identity
    SQUARE = mybir.ActivationFunctionType.Square

    assert box[0] == box[1] == box[2], "kernel assumes a cubic box"
    bx = float(box[0])

    nc = bacc.Bacc()
    pin_d = nc.declare_dram_parameter("pin", [P, 3, N + 1], F32, isOutput=False)
    out_d = nc.declare_dram_parameter("out", [P, 5, N], OUTDT, isOutput=True)

    pool_eng = nc.gpsimd if cfg["use_pool"] else nc.vector

    with TileContext(nc) as tc:
        with (
            tc.tile_pool(name="constp", bufs=1) as cpool,
            tc.tile_pool(name="pinp", bufs=2) as ppool,
            tc.tile_pool(name="work", bufs=cfg["work_bufs"]) as wpool,
            tc.tile_pool(name="outp", bufs=cfg["out_bufs"]) as opool,
        ):
            if cfg["use_pool"]:
                C3 = cpool.tile([P, 1], F32)
                nc.vector.memset(C3[:], 3.0)
            if cfg["hoist_in"]:
                pin = ppool.tile([P, 3, N + 1], F32, tag="pin")
                nc.gpsimd.dma_start(out=pin[:], in_=pin_d[:])
            for rep in range(repeat):
                if not cfg["hoist_in"]:
                    pin = ppool.tile([P, 3, N + 1], F32, tag="pin")
                    # input on the SWDGE (gpsimd) queue: keeps the HWDGE
                    # lanes free for output DMAs, overlapping previous rep
                    nc.gpsimd.dma_start(out=pin[:], in_=pin_d[:])
                q = [pin[:, c, N:N + 1] for c in range(3)]

                for ch in range(nch):
                    js = slice(ch * C, (ch + 1) * C)
                    D = wpool.tile([P, 3, C], F32, tag="D")
                    T = wpool.tile([P, 3, C], F32, tag="T")
                    SG = wpool.tile([P, 3, C], F32, tag="SG")
                    SQ = wpool.tile([P, 3, C], F32, tag="SQ")
                    W = wpool.tile([P, 4, C], F32, tag="W")
                    S0 = wpool.tile([P, C], F32, tag="S0")
                    U = wpool.tile([P, C], F32, tag="U")
                    V = wpool.tile([P, C], F32, tag="V")
                    if cfg["use_pool"]:
                        M = wpool.tile([P, C], F32, tag="M")
                        V2 = wpool.tile([P, C], F32, tag="V2")
                    outt = opool.tile([P, 5, C], OUTDT, tag="outt")

                    # d_c = q_c - p_c  (ACT Identity: affine, small values)
                    for c in range(3):
                        if cfg["d_dve"]:
                            nc.vector.tensor_scalar(
                                out=D[:, c, :], in0=pin[:, c, js],
                                scalar1=q[c], scalar2=-1.0,
                                op0=SUB, op1=MULT)
                        else:
                            nc.scalar.activation(
                                out=D[:, c, :], in_=pin[:, c, js],
                                func=IDENT, bias=q[c], scale=-1.0)
                    # t = d/box + MAGIC: f32 addition rounds to
                    # round(d/box) + MAGIC exactly (DVE ALU, bit-exact)
                    nc.vector.tensor_scalar(
                        out=T[:], in0=D[:], scalar1=1.0 / bx, scalar2=MAGIC,
                        op0=MULT, op1=ADD)
                    # sigma = t - MAGIC in {-1, 0, +1}
                    nc.vector.tensor_scalar(
                        out=SG[:], in0=T[:], scalar1=MAGIC, scalar2=None,
                        op0=SUB)
                    # non-periodic axes: only the zero shift is allowed
                    for c in range(3):
                        if not pbc_tuple[c]:
                            nc.vector.memset(SG[:, c, :], 0.0)
                    # w = d - box*sigma  (min-image displacement)
                    nc.vector.scalar_tensor_tensor(
                        out=W[:, 0:3, :], in0=SG[:], scalar=-bx, in1=D[:],
                        op0=MULT, op1=ADD)
                    # squares on ACT
                    nc.scalar.activation(out=SQ[:], in_=W[:, 0:3, :],
                                         func=SQUARE)
                    # sod = (sqx + sqy) + sqz — the one Pool-sized job
                    sod_eng = nc.gpsimd if cfg["pool_sod"] else pool_eng
                    sod_eng.tensor_tensor(
                        out=S0[:], in0=SQ[:, 0, :], in1=SQ[:, 1, :], op=ADD)
                    sod_eng.tensor_tensor(
                        out=W[:, 3, :], in0=S0[:], in1=SQ[:, 2, :], op=ADD)

                    # first-writer memset absorbs the DMA slot-recycling wait
                    nc.vector.memset(outt[:, 0, 0:1], 0.0)
                    if cfg["use_pool"]:
                        # masked planes = m * [wx, wy, wz, sod] (one Pool TT)
                        nc.vector.tensor_scalar(
                            out=M[:], in0=W[:, 3, :], scalar1=RC2,
                            scalar2=None, op0=ISLT)
                        pool_eng.tensor_tensor(
                            out=outt[:, 0:4, :], in0=W[:, 0:4, :],
                            in1=M[:].unsqueeze(1).broadcast_to([P, 4, C]),
                            op=MULT)
                        nc.vector.tensor_scalar(
                            out=U[:], in0=SG[:, 0, :], scalar1=9.0,
                            scalar2=13.0, op0=MULT, op1=ADD)
                        pool_eng.tensor_tensor(
                            out=V[:], in0=SG[:, 1, :],
                            in1=C3[:].broadcast_to([P, C]), op=MULT)
                        pool_eng.tensor_tensor(
                            out=V2[:], in0=V[:], in1=SG[:, 2, :], op=ADD)
                        pool_eng.tensor_tensor(
                            out=outt[:, 4, :], in0=U[:], in1=V2[:], op=ADD)
                    else:
                        # fused masked planes: (sod < rc^2) * plane, one STT
                        nc.vector.scalar_tensor_tensor(
                            out=outt[:, 0:4, :],
                            in0=W[:, 3:4, :].broadcast_to([P, 4, C]),
                            scalar=RC2, in1=W[:, 0:4, :],
                            op0=ISLT, op1=MULT)
                        # sidx = (9*sx + 13) + (3*sy + sz)
                        nc.vector.tensor_scalar(
                            out=U[:], in0=SG[:, 0, :], scalar1=9.0,
                            scalar2=13.0, op0=MULT, op1=ADD)
                        nc.vector.scalar_tensor_tensor(
                            out=V[:], in0=SG[:, 1, :], scalar=3.0,
                            in1=SG[:, 2, :], op0=MULT, op1=ADD)
                        nc.vector.tensor_tensor(
                            out=outt[:, 4, :], in0=U[:], in1=V[:], op=ADD)

                    nc.sync.dma_start(out=out_d[:, :, js], in_=outt[:])
    nc.finalize()
    return nc


def _prep(pos_cel, cel_mat, pbc):
    """Host-side prep shared by kernel() and bench.py."""
    pos_cel = np.asarray(pos_cel)
    cel_mat = np.asarray(cel_mat, dtype=np.float32)
    pbc = np.asarray(pbc)
    B = pos_cel.shape[0]
    assert pos_cel.shape == (B, N, 3), pos_cel.shape
    assert B == 1

    pos = pos_cel[0].astype(np.float32) @ cel_mat[0]
    pos = pos.astype(np.float32)
    off = cel_mat[0] - np.diag(np.diag(cel_mat[0]))
    assert np.all(off == 0), "kernel assumes a diagonal cell matrix"
    box = tuple(float(cel_mat[0][c, c]) for c in range(3))
    pbc_tuple = tuple(bool(x) for x in pbc[0])

    in_maps = []
    for k in range(NCORES):
        pin = np.empty((P, 3, N + 1), dtype=np.float32)
        pin[:, :, :N] = pos.T[None]
        pin[:, :, N] = pos[k * P:(k + 1) * P]
        in_maps.append({"pin": pin})
    return box, pbc_tuple, in_maps


def _scatter(planes):
    """planes: [N, 5, N] device output -> full [1, N, N, S, 4] f32."""
    vals = np.ascontiguousarray(
        planes[:, 0:4, :].transpose(0, 2, 1), dtype=np.float32)  # [N, N, 4]
    sidx = np.clip(planes[:, 4, :].astype(np.int32), 0, S - 1)   # [N, N]
    out = np.zeros((N * N * S, 4), np.float32)
    ii = np.arange(N, dtype=np.int64)
    lin = (ii[:, None] * N + ii[None, :]) * S + sidx
    out[lin.reshape(-1)] = vals.reshape(N * N, 4)
    return out.reshape(1, N, N, S, 4)


def kernel(pos_cel, cel_mat, pbc):
    global LAST_RESULT
    from concourse.bass_utils import run_bass_kernel_spmd

    box, pbc_tuple, in_maps = _prep(pos_cel, cel_mat, pbc)

    key = (box, pbc_tuple)
    if key not in _CACHE:
        _CACHE[key] = _build(box, pbc_tuple)
    nc = _CACHE[key]

    res = run_bass_kernel_spmd(nc, in_maps, list(range(NCORES)), trace=TRACE)
    LAST_RESULT = res

    planes = np.concatenate(
        [np.asarray(res.results[k]["out"], dtype=np.float32)
         for k in range(NCORES)], axis=0)              # [N, 5, N]
    return _scatter(planes)
